# revision 1
# baseline (speedup 1.0000x reference)
"""BBoxEncoder Trainium2 kernel.

Per ray r, BVH level l (8 levels), the reference gathers an embedding row
f = bbox_emb[history[r, l]] (8 corners x 32 dims), normalizes the ray's 16
sample points into the node's AABB, builds trilinear corner weights
w[p, c] and emits feat[r, l, p, d] = sum_c w[p, c] * f[c, d].

Strategy (data-parallel over rays, 8 NeuronCores):
  - shard inp/history along axis 0, replicate bbox_emb + node AABBs
  - per 128-ray SBUF tile: indirect-DMA gather of emb rows + AABB rows,
    DVE computes normalized coords, corner weight factors, and the
    8-corner multiply-accumulate; contiguous DMA of the (128, 4096) output
    tile back to HBM.

kernel(**inputs) takes the FULL unsharded inputs and returns the FULL
(32768, 4096) float32 output.
"""

import numpy as np
from contextlib import ExitStack

import concourse.bass as bass
import concourse.tile as tile
from concourse import bacc, mybir
from concourse.bass import IndirectOffsetOnAxis, ts
from concourse.bass_utils import run_bass_kernel_spmd

F32 = mybir.dt.float32
I32 = mybir.dt.int32
AL = mybir.AluOpType

ENC_DEPTH = 8
N_POINTS = 16
ENC_DIM = 32

N_CORES = 8
N_RAYS = 32768
N_NODES = 65536

# corner order used on-device: c = bx*4 + by*2 + bz (x-bit major).
# reference order (torch chunk order): 000,100,010,001,101,011,110,111
# as (bx, by, bz) bit tuples.
_REF_CORNERS = [
    (0, 0, 0), (1, 0, 0), (0, 1, 0), (0, 0, 1),
    (1, 0, 1), (0, 1, 1), (1, 1, 0), (1, 1, 1),
]
# PERM[c_lex] = index of corner c_lex=(bx,by,bz) in the reference order.
PERM = [0] * 8
for _i, (_bx, _by, _bz) in enumerate(_REF_CORNERS):
    PERM[_bx * 4 + _by * 2 + _bz] = _i
# emb columns are permuted on the host so that device corner index c_lex
# reads emb block c_lex directly.


def _emit(ctx: ExitStack, tc, io, n_shard, n_levels):
    """Emit the per-core program. io: dict of DRAM tensor handles."""
    nc = tc.nc
    P = 128
    L = n_levels
    n_tiles = n_shard // P
    PD = N_POINTS * ENC_DIM  # 512
    OUT_W = ENC_DEPTH * PD  # 4096

    inp_d = io["inp"].ap()        # (n_shard, 48)
    hist_d = io["hist"].ap()      # (n_shard, 8) int32
    nodes_d = io["nodes"]         # (N_NODES, 6)  [min | max]
    emb_d = io["emb"]             # (N_NODES, 256) corner-permuted
    out_d = io["out"].ap()        # (n_shard, 4096)

    ld = ctx.enter_context(tc.tile_pool(name="ld", bufs=3))
    gat = ctx.enter_context(tc.tile_pool(name="gat", bufs=3))
    wrk = ctx.enter_context(tc.tile_pool(name="wrk", bufs=2))
    acc_p = ctx.enter_context(tc.tile_pool(name="acc", bufs=3))

    for i in range(n_tiles):
        inp_t = ld.tile([P, 48], F32, tag="inp")
        nc.sync.dma_start(inp_t[:], inp_d[ts(i, P), :])
        hist_t = ld.tile([P, 8], I32, tag="hist")
        nc.sync.dma_start(hist_t[:], hist_d[ts(i, P), :])

        f_t = gat.tile([P, L * 256], F32, tag="f")
        nd_t = gat.tile([P, L * 6], F32, tag="nd")
        for l in range(L):
            nc.gpsimd.indirect_dma_start(
                out=nd_t[:, l * 6:(l + 1) * 6],
                out_offset=None,
                in_=nodes_d.ap(),
                in_offset=IndirectOffsetOnAxis(ap=hist_t[:, l:l + 1], axis=0),
            )
            nc.gpsimd.indirect_dma_start(
                out=f_t[:, l * 256:(l + 1) * 256],
                out_offset=None,
                in_=emb_d.ap(),
                in_offset=IndirectOffsetOnAxis(ap=hist_t[:, l:l + 1], axis=0),
            )

        nd_v = nd_t[:].rearrange("q (l e) -> q l e", e=6)  # (P, L, 6)

        # extent = max - min; extent==0 -> 0.5; inv = 1/extent
        ext_t = wrk.tile([P, L * 3], F32, tag="ext")
        ext_v = ext_t[:].rearrange("q (l e) -> q l e", e=3)
        nc.vector.tensor_tensor(
            out=ext_v, in0=nd_v[:, :, 3:6], in1=nd_v[:, :, 0:3], op=AL.subtract
        )
        msk_t = wrk.tile([P, L * 3], F32, tag="msk")
        nc.vector.tensor_scalar(
            out=msk_t[:], in0=ext_t[:], scalar1=0.0, scalar2=0.5,
            op0=AL.is_equal, op1=AL.mult,
        )
        nc.vector.tensor_tensor(
            out=ext_t[:], in0=ext_t[:], in1=msk_t[:], op=AL.add
        )
        inv_t = wrk.tile([P, L * 3], F32, tag="inv")
        nc.vector.reciprocal(inv_t[:], ext_t[:])
        inv_v = inv_t[:].rearrange("q (l e) -> q l e", e=3)

        # x[q, l, p, e] = clip((inp[q, p, e] - nmin[q, l, e]) * inv[q, l, e])
        x_t = wrk.tile([P, L * 48], F32, tag="x")
        x_v = x_t[:].rearrange("q (l p e) -> q l p e", p=N_POINTS, e=3)
        inp_b = (
            inp_t[:].rearrange("q (p e) -> q p e", e=3)
            .unsqueeze(1).to_broadcast([P, L, N_POINTS, 3])
        )
        nmin_b = nd_v[:, :, 0:3].unsqueeze(2).to_broadcast([P, L, N_POINTS, 3])
        nc.vector.tensor_tensor(out=x_v, in0=inp_b, in1=nmin_b, op=AL.subtract)
        inv_b = inv_v.unsqueeze(2).to_broadcast([P, L, N_POINTS, 3])
        nc.vector.tensor_tensor(out=x_v, in0=x_v, in1=inv_b, op=AL.mult)
        nc.vector.tensor_scalar(
            out=x_t[:], in0=x_t[:], scalar1=0.0, scalar2=1.0,
            op0=AL.max, op1=AL.min,
        )

        # factor tile ft[q, axis, l, s, p]: s=0 -> 1-t, s=1 -> t
        ft_t = wrk.tile([P, 3 * L * 2 * N_POINTS], F32, tag="ft")
        ft_v = ft_t[:].rearrange(
            "q (a l s p) -> q a l s p", a=3, l=L, s=2, p=N_POINTS
        )
        # walk (l, p, a) on both sides; out strided, in contiguous
        x_w = x_v.transpose([0, 3, 1, 2])            # (P, 3, L, 16) view
        nc.vector.tensor_scalar(
            out=ft_v[:, :, :, 0, :].transpose([0, 2, 3, 1]),
            in0=x_w.transpose([0, 2, 3, 1]),
            scalar1=-1.0, scalar2=1.0, op0=AL.mult, op1=AL.add,
        )
        nc.vector.tensor_copy(
            out=ft_v[:, :, :, 1, :].transpose([0, 2, 3, 1]),
            in_=x_w.transpose([0, 2, 3, 1]),
        )

        # wxy[q, l, bx, by, p] then w[q, l, bx, by, bz, p]
        # (ISA allows at most 3 free dims per DVE AP -> split by corner bits)
        wxy_t = wrk.tile([P, L * 4 * N_POINTS], F32, tag="wxy")
        wxy_v = wxy_t[:].rearrange(
            "q (l x y p) -> q l x y p", x=2, y=2, p=N_POINTS
        )
        for bx in range(2):
            nc.vector.tensor_tensor(
                out=wxy_v[:, :, bx],
                in0=ft_v[:, 0, :, bx, :].unsqueeze(2)
                    .to_broadcast([P, L, 2, N_POINTS]),
                in1=ft_v[:, 1],
                op=AL.mult,
            )
        w_t = wrk.tile([P, L * 8 * N_POINTS], F32, tag="w")
        w_v = w_t[:].rearrange(
            "q (l x y z p) -> q l x y z p", x=2, y=2, z=2, p=N_POINTS
        )
        for bx in range(2):
            for by in range(2):
                nc.vector.tensor_tensor(
                    out=w_v[:, :, bx, by],
                    in0=wxy_v[:, :, bx, by, :].unsqueeze(2)
                        .to_broadcast([P, L, 2, N_POINTS]),
                    in1=ft_v[:, 2],
                    op=AL.mult,
                )
        w_c = w_t[:].rearrange("q (l c p) -> q l c p", c=8, p=N_POINTS)
        f_c = f_t[:].rearrange("q (l c d) -> q l c d", c=8, d=ENC_DIM)

        # acc[q, l, p, d] = sum_c w[q, l, c, p] * f[q, l, c, d]
        acc_t = acc_p.tile([P, OUT_W], F32, tag="acc")
        acc_v = acc_t[:, : L * PD].rearrange(
            "q (l p d) -> q l p d", p=N_POINTS, d=ENC_DIM
        )
        tmp_t = wrk.tile([P, L * PD], F32, tag="tmp")
        tmp_v = tmp_t[:].rearrange(
            "q (l p d) -> q l p d", p=N_POINTS, d=ENC_DIM
        )
        for c in range(8):
            dst = acc_v if c == 0 else tmp_v
            nc.vector.tensor_tensor(
                out=dst,
                in0=w_c[:, :, c, :].unsqueeze(3)
                    .to_broadcast([P, L, N_POINTS, ENC_DIM]),
                in1=f_c[:, :, c, :].unsqueeze(2)
                    .to_broadcast([P, L, N_POINTS, ENC_DIM]),
                op=AL.mult,
            )
            if c > 0:
                nc.vector.tensor_tensor(
                    out=acc_v, in0=acc_v, in1=tmp_v, op=AL.add
                )
        if L < ENC_DEPTH:
            nc.gpsimd.memset(acc_t[:, L * PD:], 0.0)

        nc.sync.dma_start(out_d[ts(i, P), :], acc_t[:])


def build_program(n_shard, n_nodes, n_levels):
    nc = bacc.Bacc(
        "TRN2", target_bir_lowering=False, debug=False, enable_asserts=False
    )
    io = {
        "inp": nc.dram_tensor("inp", [n_shard, 48], F32, kind="ExternalInput"),
        "hist": nc.dram_tensor("hist", [n_shard, 8], I32, kind="ExternalInput"),
        "nodes": nc.dram_tensor("nodes", [n_nodes, 6], F32, kind="ExternalInput"),
        "emb": nc.dram_tensor("emb", [n_nodes, 256], F32, kind="ExternalInput"),
        "out": nc.dram_tensor(
            "out", [n_shard, ENC_DEPTH * N_POINTS * ENC_DIM], F32,
            kind="ExternalOutput",
        ),
    }
    with tile.TileContext(nc) as tc, ExitStack() as ctx:
        _emit(ctx, tc, io, n_shard, n_levels)
    nc.compile()
    return nc


_CACHE = {}


def _get_program(n_shard, n_nodes, n_levels):
    key = (n_shard, n_nodes, n_levels)
    if key not in _CACHE:
        _CACHE[key] = build_program(n_shard, n_nodes, n_levels)
    return _CACHE[key]


def make_in_maps(inp, history, bbox_emb, nodes_min, nodes_max, n_cores=N_CORES):
    """Host-side marshalling: shard rays, permute emb corners, pack AABBs."""
    n_rays = inp.shape[0]
    shard = n_rays // n_cores
    inp_f = np.ascontiguousarray(
        inp.reshape(n_rays, 48).astype(np.float32, copy=False)
    )
    hist8 = np.ascontiguousarray(history[:, :ENC_DEPTH].astype(np.int32, copy=False))
    nodes = np.ascontiguousarray(
        np.concatenate(
            [nodes_min.astype(np.float32, copy=False),
             nodes_max.astype(np.float32, copy=False)], axis=1
        )
    )
    n_nodes = nodes.shape[0]
    emb_p = np.ascontiguousarray(
        bbox_emb.astype(np.float32, copy=False)
        .reshape(n_nodes, 8, ENC_DIM)[:, PERM, :]
        .reshape(n_nodes, 8 * ENC_DIM)
    )
    in_maps = []
    for c in range(n_cores):
        sl = slice(c * shard, (c + 1) * shard)
        in_maps.append({
            "inp": inp_f[sl],
            "hist": hist8[sl],
            "nodes": nodes,
            "emb": emb_p,
        })
    return in_maps, shard, n_nodes


def kernel(inp, history, depth, bbox_emb, nodes_min, nodes_max):
    inp = np.asarray(inp)
    history = np.asarray(history)
    depth = np.asarray(depth)
    bbox_emb = np.asarray(bbox_emb)
    nodes_min = np.asarray(nodes_min)
    nodes_max = np.asarray(nodes_max)

    n_rays = inp.shape[0]
    n_levels = int(min(int(depth.max()), ENC_DEPTH)) if depth.size else 0
    in_maps, shard, n_nodes = make_in_maps(
        inp, history, bbox_emb, nodes_min, nodes_max
    )
    if n_levels <= 0:
        return np.zeros((n_rays, ENC_DEPTH * N_POINTS * ENC_DIM), np.float32)

    nc = _get_program(shard, n_nodes, n_levels)
    res = run_bass_kernel_spmd(nc, in_maps, core_ids=list(range(N_CORES)))
    out = np.concatenate([r["out"] for r in res.results], axis=0)
    return out


_FAST = {}


def _run_fast(nc, in_maps):
    """Sharded PJRT runner that avoids the 2x full-output H2D of the generic
    path: replicated tensors ship once (P(None)) and the donated zero output
    buffers are created on-device inside the jit."""
    import jax
    import jax.numpy as jnp
    from jax.sharding import Mesh, PartitionSpec as P
    from jax.experimental.shard_map import shard_map
    from concourse import bass2jax

    n_cores = len(in_maps)
    key = id(nc)
    if key not in _FAST:
        bass2jax.install_neuronx_cc_hook()
        part_name = nc.partition_id_tensor.name if nc.partition_id_tensor else None
        in_names, out_names, out_avals = [], [], []
        for alloc in nc.m.functions[0].allocations:
            if not isinstance(alloc, mybir.MemoryLocationSet):
                continue
            name = alloc.memorylocations[0].name
            if alloc.kind == "ExternalInput":
                if name != part_name:
                    in_names.append(name)
            elif alloc.kind == "ExternalOutput":
                out_names.append(name)
                out_avals.append(jax.core.ShapedArray(
                    tuple(alloc.tensor_shape), mybir.dt.np(alloc.dtype)))
        repl = {"nodes", "emb"}
        all_names = list(in_names) + list(out_names)
        if part_name is not None:
            all_names.append(part_name)

        def _body(*args):
            operands = list(args)
            operands += [jnp.zeros(a.shape, a.dtype) for a in out_avals]
            if part_name is not None:
                operands.append(bass2jax.partition_id_tensor())
            return tuple(bass2jax._bass_exec_p.bind(
                *operands,
                out_avals=tuple(out_avals),
                in_names=tuple(all_names),
                out_names=tuple(out_names),
                lowering_input_output_aliases=(),
                sim_require_finite=True,
                sim_require_nnan=True,
                nc=nc,
            ))

        devices = jax.devices()[:n_cores]
        mesh = Mesh(np.asarray(devices), ("core",))
        in_specs = tuple(
            P() if nm in repl else P("core") for nm in in_names
        )
        sharded = jax.jit(shard_map(
            _body, mesh=mesh, in_specs=in_specs,
            out_specs=(P("core"),) * len(out_names), check_rep=False,
        ))
        _FAST[key] = (sharded, in_names, out_names, repl, mesh)

    sharded, in_names, out_names, repl, mesh = _FAST[key]
    args = []
    for nm in in_names:
        if nm in repl:
            args.append(in_maps[0][nm])
        else:
            args.append(np.concatenate(
                [np.asarray(m[nm]) for m in in_maps], axis=0))
    outs = sharded(*args)
    return np.asarray(outs[out_names.index("out")])

